# revision 56
# baseline (speedup 1.0000x reference)
"""Trainium2 Bass kernel for 12-head causal multi-head attention.

Problem: B=8, T=1024, C=768, H=12, HS=64, fp32 reference.
Sharding: data-parallel over batch — core b computes batch element b.

Per-core algorithm (everything PE-friendly, no on-chip transposes):
  - host passes x[b] TRANSPOSED (xT [C, T]) plus repacked weights in
    bfloat16 (PSUM accumulation stays fp32; final rel err ~4e-3, well
    inside the 2e-2 gate; bf16 halves DMA bytes, removes the fp32r
    min-free-size matmul penalty, and unlocks the DVE 2x mode)
  - qT/kT computed in [head*64, T] layout (head pairs packed into 128
    partitions), v in token-major [T, head*65] layout with a ones column
    appended per head (the ones column makes the PV matmul emit softmax
    row-sums for free)
  - scores are computed TRANSPOSED (s on partitions, t on free dim):
    exp needs no max subtraction (|score| <= ~6 for this data) so softmax
    becomes a single scalar-engine Exp over a head PAIR per instruction,
    and causal masking is a triangular 0/1 mask multiply on the diagonal
    128-col block only; columns left of the diagonal are never computed
    and diagonal blocks are exactly 128 wide (no bf16 width penalty)
  - PV produces outT [d, T] per head pair, which is exactly the lhsT
    layout the output projection needs; per-query normalization takes a
    reciprocal of the PSUM row-sum row per head into a staging row, then
    BOUNCES it through an internal DRAM scratch: the return DMA
    replicates it across the partition dim (engines cannot partition-
    broadcast, and SBUF-source DMAs need nonzero partition stride), so
    normalization is one in-place fast-mode DVE multiply per head and no
    PE broadcast matmul is needed
  - y is produced in bf16 (host upcasts) halving output DMA bytes; the
    bias add+eviction is split in column halves so the store DMA
    overlaps the add (shorter drain tail)
  - emission is software-pipelined: qkT of pair p+1 and the second half
    of the v projection are emitted inside pair p's attention so the
    in-order PE queue always has independent work to fill exp-wait
    bubbles; a 2-deep scores/exp lookahead hides the pair-boundary
    pipeline refill, and dummy warmup matmuls ramp the PE pstate clock
    while the first input DMAs are in flight
"""

import os
import numpy as np

B, T, C = 8, 1024, 768
H, HS = 12, 64
NPAIR = 6  # head pairs (2 heads of 64 -> 128 partitions)
NCK = 6    # contraction chunks of 128 over C
NT = 8     # token tiles of 128

LAST_EXEC_NS = None
LAST_RESULTS = None

_cached_nc = None


def _build_nc():
    import concourse.bass as bass
    import concourse.mybir as mybir
    import concourse.tile as tile
    from concourse import bacc
    from concourse.masks import make_upper_triangular

    f32 = mybir.dt.float32
    f32r = mybir.dt.float32r
    bf16 = mybir.dt.bfloat16
    AF = mybir.ActivationFunctionType

    nc = bacc.Bacc("TRN2", target_bir_lowering=False, debug=False, num_devices=8)

    xT_d = nc.dram_tensor("xT", [C, T], bf16, kind="ExternalInput")
    wqk_d = nc.dram_tensor(
        "wqk", [NPAIR, 128, NCK, 256], bf16, kind="ExternalInput"
    )
    wv_d = nc.dram_tensor("wv", [C, C], bf16, kind="ExternalInput")
    wp_d = nc.dram_tensor("wp", [NPAIR, 128, C], bf16, kind="ExternalInput")
    bp_d = nc.dram_tensor("bp", [C], f32, kind="ExternalInput")
    y_d = nc.dram_tensor("y", [T, C], bf16, kind="ExternalOutput")
    # DRAM bounce buffer for the 1/rowsum rows: DMA can replicate across
    # partitions only from DRAM (SBUF APs need nonzero partition step)
    rscr_d = nc.dram_tensor("rscr", [NPAIR * 2, 2 * 512], bf16,
                            kind="Internal")

    with tile.TileContext(nc) as tc:
        with (
            tc.tile_pool(name="const", bufs=1) as const,
            tc.tile_pool(name="work", bufs=2) as work,
            tc.tile_pool(name="ppool", bufs=4) as ppool,
            tc.tile_pool(name="opool", bufs=1) as opool,
            tc.tile_pool(name="ps1", bufs=2, space="PSUM") as ps1,
        ):
            # ---------- resident inputs / constants ----------
            # wqkt for pair 0 goes on the (otherwise idle) gpsimd queue so
            # its transfer runs parallel to the xT loads
            wqkt0 = work.tile([128, NCK, 256], bf16, tag="wqkt", bufs=3,
                              name="wqkt")
            nc.gpsimd.dma_start(out=wqkt0, in_=wqk_d[0])
            xts = []
            for i in range(NCK):
                xt = const.tile([128, T], bf16, tag=f"xt{i}", name=f"xt{i}")
                eng = (nc.sync, nc.scalar, nc.gpsimd)[i % 3]
                eng.dma_start(out=xt, in_=xT_d[i * 128:(i + 1) * 128, :])
                xts.append(xt)
            wvts = []
            for i in range(NCK):
                wvt = const.tile([128, C], bf16, tag=f"wv{i}", name=f"wv{i}")
                eng = nc.scalar if i % 2 == 0 else nc.sync
                eng.dma_start(out=wvt, in_=wv_d[i * 128:(i + 1) * 128, :])
                wvts.append(wvt)
            U = const.tile([128, 128], bf16)
            with nc.allow_low_precision(reason="0/1 causal mask is exact"):
                make_upper_triangular(nc, U[:, :], val=1.0, diag=True)
            ones_f = const.tile([128, 12], bf16)
            nc.vector.memset(ones_f, 1.0)
            # PE pstate warmup: the tensor engine clock ramps with continuous
            # busy time, so burn rows on a dummy tile while the input DMAs
            # are still in flight
            wtile = const.tile([128, 512], bf16, name="wtile")
            nc.vector.memset(wtile, 0.0)
            wps = ps1.tile([128, 512], f32, tag="po", bufs=4, name="wps")
            for wi in range(8):
                nc.tensor.matmul(
                    wps,
                    lhsT=wtile[:, 0:128],
                    rhs=wtile,
                    start=(wi == 0),
                    stop=(wi == 7),
                )


            # v in token-major layout: per k-tile, 12 heads x (64 cols of v | 1)
            v_all = const.tile([128, NT, H * 65], bf16)
            v_heads = v_all.rearrange("p k (h c) -> p k h c", h=H)
            for kt in range(NT):
                nc.vector.tensor_copy(
                    out=v_heads[:, kt, :, 64:65],
                    in_=ones_f.rearrange("p (h o) -> p h o", o=1),
                )

            outTs = [
                opool.tile([128, T], bf16, tag=f"outT{p}", name=f"outT{p}")
                for p in range(NPAIR)
            ]
            # rowsum-reciprocal staging: one [1, 1024] row per (pair, qc),
            # rotating over partition rows 0/32/64/96 (DVE writes must be
            # 32-aligned in the partition dim)
            rsb = opool.tile([128, 2 * 512], bf16, tag="rsb", name="rsb")

            # ---------- emit helpers (software-pipelined) ----------
            def emit_vproj(kts):
                with nc.named_scope("vproj"):
                    for kt in kts:
                        pv = ps1.tile([128, C], f32, tag="big2", bufs=2,
                                      name="pv")
                        for ck in range(NCK):
                            for n0, n1 in ((0, 512), (512, 768)):
                                nc.tensor.matmul(
                                    pv[:, n0:n1],
                                    lhsT=xts[ck][:, kt * 128:(kt + 1) * 128],
                                    rhs=wvts[ck][:, n0:n1],
                                    start=(ck == 0),
                                    stop=(ck == NCK - 1),
                                )
                        with nc.allow_low_precision(
                            reason="v in bf16; matmul accumulates fp32"
                        ):
                            nc.scalar.copy(
                                out=v_heads[:, kt, :, 0:64],
                                in_=pv.rearrange("p (h c) -> p h c", h=H),
                            )

            qkts = {}

            def qkT_group(p, wqkt, qT, kTt, gi):
                dst, wo = ((qT, 0), (kTt, 128))[gi // 2]
                tch = gi % 2
                with nc.named_scope(f"qk{p}"):
                    pqk = ps1.tile([128, 512], f32, tag="po", bufs=4,
                                   name="pqk")
                    for ck in range(NCK):
                        nc.tensor.matmul(
                            pqk,
                            lhsT=wqkt[:, ck, wo:wo + 128],
                            rhs=xts[ck][:, tch * 512:(tch + 1) * 512],
                            start=(ck == 0),
                            stop=(ck == NCK - 1),
                        )
                    with nc.allow_low_precision(
                        reason="q/k in bf16; scores err ~0.6%"
                    ):
                        if tch == 0:
                            nc.scalar.copy(out=dst[:, 0:512], in_=pqk)
                        else:
                            nc.vector.tensor_copy(out=dst[:, 512:1024],
                                                  in_=pqk)

            def make_qkT_fillers(p, wqkt=None):
                # allocate tiles and launch the weight DMA now; the four
                # matmul groups are emitted later, dripped into exp-wait
                # bubbles of the current pair's attention
                if wqkt is None:
                    wqkt = work.tile([128, NCK, 256], bf16, tag="wqkt",
                                     bufs=3, name="wqkt")
                    nc.sync.dma_start(out=wqkt, in_=wqk_d[p])
                qT = work.tile([128, T], bf16, tag="qT", bufs=3, name="qT")
                kTt = work.tile([128, T], bf16, tag="kTt", bufs=3, name="kTt")
                qkts[p] = (qT, kTt)
                return [
                    (lambda gi=gi: qkT_group(p, wqkt, qT, kTt, gi))
                    for gi in range(4)
                ]

            for f in make_qkT_fillers(0, wqkt0):
                f()
            fillers = []

            # ---------- phase 1: attention (qkT/vproj pipelined in) --------
            for p in range(NPAIR):
                qT, kTt = qkts.pop(p)
                if p + 1 < NPAIR:
                    fillers += make_qkT_fillers(p + 1)
                with nc.named_scope(f"att{p}"):
                    for qc in range(2):
                        nkt = 4 * (qc + 1)
                        po_pair = []
                        for hh in range(2):
                            po = ps1.tile([65, 512], f32, tag="po", bufs=4,
                                          name=f"po{hh}")
                            po_pair.append(po)
                        def scores_exp(kt):
                            # scores pair -> exp -> causal mask for one k-tile
                            ccol = max(0, 128 * kt - 512 * qc)
                            pt = ppool.tile([128, 2, 512], bf16, tag="pt",
                                            bufs=6, name="pt")
                            pscr = ps1.tile([128, 2, 512], f32, tag="big2",
                                            bufs=2, name="pscr")
                            for hh in range(2):
                                nc.tensor.matmul(
                                    pscr[:, hh, ccol:512],
                                    lhsT=kTt[hh * 64:(hh + 1) * 64,
                                             kt * 128:(kt + 1) * 128],
                                    rhs=qT[hh * 64:(hh + 1) * 64,
                                           qc * 512 + ccol:(qc + 1) * 512],
                                    start=True,
                                    stop=True,
                                )
                            with nc.allow_low_precision(
                                reason="softmax weights in bf16 (~0.4%)"
                            ):
                                nc.scalar.activation(
                                    out=pt[:, :, ccol:512],
                                    in_=pscr[:, :, ccol:512],
                                    func=AF.Exp,
                                    scale=float(HS) ** -0.5,
                                )
                                if 128 * kt >= 512 * qc:
                                    nc.gpsimd.tensor_mul(
                                        pt[:, :, ccol:ccol + 128],
                                        pt[:, :, ccol:ccol + 128],
                                        U.rearrange("p (o c) -> p o c", o=1)
                                         .broadcast_to([128, 2, 128]),
                                    )
                            return pt, ccol

                        # software-pipelined: scores/exp/mask run ahead of
                        # the PV that consumes them. For pair 0 the lookahead
                        # is deep (scores need only qT/kT) and the v
                        # projection is emitted AFTER the prefill, so exp
                        # work fills the wait for the Wv/x input DMAs.
                        look = 8 if p == 0 else 2
                        pts = {}
                        for kk in range(min(look, nkt)):
                            pts[kk] = scores_exp(kk)
                        if p == 0 and qc == 0:
                            emit_vproj(range(0, 4))
                        if p == 0 and qc == 1:
                            emit_vproj(range(4, NT))
                        for kt in range(nkt):
                            if kt + look < nkt:
                                pts[kt + look] = scores_exp(kt + look)
                            pt, ccol = pts.pop(kt)
                            for hh in range(2):
                                h = 2 * p + hh
                                nc.tensor.matmul(
                                    po_pair[hh][:, ccol:512],
                                    lhsT=v_all[:, kt, h * 65:(h + 1) * 65],
                                    rhs=pt[:, hh, ccol:512],
                                    start=(kt == 0),
                                    stop=(kt == nkt - 1),
                                )
                            if fillers and kt % 2 == 1:
                                fillers.pop(0)()
                        qsl = slice(qc * 512, (qc + 1) * 512)
                        # 1/rowsum per head (row 64 of po is the ones-column
                        # rowsum); both heads' recip rows go into one
                        # [1, 1024] staging row, bounce through DRAM, and
                        # come back replicated across the partition dim so
                        # the PSUM eviction fuses with the normalize multiply
                        pq = 2 * p + qc
                        r0 = (pq % 4) * 32
                        with nc.allow_low_precision(
                            reason="1/rowsum in bf16 costs ~0.4% uniform "
                                   "per-(head,query) scale error"
                        ):
                            for hh in range(2):
                                nc.vector.reciprocal(
                                    out=rsb[r0:r0 + 1,
                                            hh * 512:(hh + 1) * 512],
                                    in_=po_pair[hh][64:65, :],
                                )
                        nc.sync.dma_start(
                            out=rscr_d[pq], in_=rsb[r0:r0 + 1, :]
                        )
                        # bounce back replicated across partitions (bf16
                        # end-to-end so no cast queue is needed and the
                        # in-place normalize multiply runs in DVE fast mode)
                        prsb = work.tile([128, 512], bf16, tag="prsb",
                                         bufs=3, name="prsb")
                        nc.gpsimd.dma_start(
                            out=prsb,
                            in_=bass.AP(
                                tensor=rscr_d,
                                offset=pq * 1024,
                                ap=[[512, 2], [0, 64], [1, 512]],
                            ),
                        )
                        with nc.allow_low_precision(
                            reason="attention output in bf16 (~0.4%)"
                        ):
                            # evict PSUM immediately (frees the po bufs for
                            # the pipelined qk groups), normalize in place
                            # once the replicated recip row lands
                            for hh in range(2):
                                nc.vector.tensor_copy(
                                    out=outTs[p][hh * 64:(hh + 1) * 64, qsl],
                                    in_=po_pair[hh][0:64, :],
                                )
                            for hh in range(2):
                                nc.vector.tensor_mul(
                                    outTs[p][hh * 64:(hh + 1) * 64, qsl],
                                    outTs[p][hh * 64:(hh + 1) * 64, qsl],
                                    prsb[hh * 64:(hh + 1) * 64, :],
                                )

            for f in fillers:
                f()
            fillers = []

            # ---------- weights for proj (loads overlap phase 1) ----------
            wpts = []
            for pp in range(NPAIR):
                wpt = const.tile([128, C], bf16, tag=f"wp{pp}", name=f"wp{pp}")
                nc.gpsimd.dma_start(out=wpt, in_=wp_d[pp])
                wpts.append(wpt)
            bias_t = const.tile([128, C], f32)
            nc.gpsimd.dma_start(
                out=bias_t,
                in_=bass.AP(tensor=bp_d, offset=0, ap=[[0, 128], [1, C]]),
            )

            # ---------- phase 2: output projection ----------
            with nc.named_scope("proj"):
                for tt in range(NT):
                    py = ps1.tile([128, C], f32, tag="big2", bufs=2, name="py")
                    for p in range(NPAIR):
                        for n0, n1 in ((0, 512), (512, 768)):
                            nc.tensor.matmul(
                                py[:, n0:n1],
                                lhsT=outTs[p][:, tt * 128:(tt + 1) * 128],
                                rhs=wpts[p][:, n0:n1],
                                start=(p == 0),
                                stop=(p == NPAIR - 1),
                            )
                    ysb = work.tile([128, C], bf16, tag="ysb", bufs=3,
                                    name="ysb")
                    # two column halves so the y DMA starts while the second
                    # half of the bias add still runs (shorter drain tail)
                    with nc.allow_low_precision(
                        reason="y returned in bf16 (~0.2%); gate is 2e-2"
                    ):
                        for c0, c1 in ((0, 384), (384, C)):
                            nc.vector.tensor_add(
                                ysb[:, c0:c1], py[:, c0:c1], bias_t[:, c0:c1]
                            )
                            nc.sync.dma_start(
                                out=y_d[tt * 128:(tt + 1) * 128, c0:c1],
                                in_=ysb[:, c0:c1],
                            )

    nc.compile()
    return nc


def get_nc():
    global _cached_nc
    if _cached_nc is None:
        _cached_nc = _build_nc()
    return _cached_nc


def _host_pack(inputs):
    import ml_dtypes

    bf16 = ml_dtypes.bfloat16
    x = np.asarray(inputs["x"], dtype=np.float32)
    Wq = np.asarray(inputs["Wq"], dtype=np.float32)
    Wk = np.asarray(inputs["Wk"], dtype=np.float32)
    Wv = np.asarray(inputs["Wv"], dtype=np.float32)
    Wproj = np.asarray(inputs["Wproj"], dtype=np.float32)
    bproj = np.asarray(inputs["bproj"], dtype=np.float32)

    Wq2 = Wq.transpose(1, 0, 2).reshape(C, C)  # [c, h*HS]
    Wk2 = Wk.transpose(1, 0, 2).reshape(C, C)
    wqk = np.stack(
        [
            np.concatenate(
                [
                    Wq2[:, p * 128:(p + 1) * 128],
                    Wk2[:, p * 128:(p + 1) * 128],
                ],
                axis=1,
            )
            for p in range(NPAIR)
        ]
    )  # [6, 768(c), 256]
    # -> [6, 128(kp), 6(ck), 256] so the on-chip tile loads contiguously
    wqk = np.ascontiguousarray(
        wqk.reshape(NPAIR, NCK, 128, 256).transpose(0, 2, 1, 3)
    ).astype(bf16)
    wv = np.ascontiguousarray(Wv.transpose(1, 0, 2).reshape(C, C)).astype(bf16)
    wp = np.ascontiguousarray(Wproj.T.reshape(NPAIR, 128, C)).astype(bf16)
    shared = {"wqk": wqk, "wv": wv, "wp": wp, "bp": bproj}
    in_maps = [
        dict(shared, xT=np.ascontiguousarray(x[b].T).astype(bf16))
        for b in range(B)
    ]
    return in_maps


def kernel(**inputs):
    global LAST_EXEC_NS, LAST_RESULTS
    from concourse.bass_utils import run_bass_kernel_spmd

    nc = get_nc()
    in_maps = _host_pack(inputs)
    trace = bool(int(os.environ.get("KERNEL_TRACE", "0")))
    res = run_bass_kernel_spmd(
        nc, in_maps, core_ids=list(range(B)), trace=trace
    )
    LAST_EXEC_NS = res.exec_time_ns
    LAST_RESULTS = res
    y = np.stack([res.results[b]["y"] for b in range(B)])
    return y.astype(np.float32)
